# revision 1
# baseline (speedup 1.0000x reference)
"""ColBERT MaxSim contrastive loss on 8 Trainium2 NeuronCores.

Sharding: doc-parallel (each core scores ALL 64*32 query tokens against its
8-doc shard = 8192 doc tokens). Per core:
  - matmul  qT[128, 2048].T @ dT[128, 8192] -> scores, tiled 128x512 into PSUM
    (fp32r: full-rate streaming)
  - max over doc tokens s (1024 per doc): ScalarE copies the odd 512-half of
    each doc PSUM->SBUF while VectorE does a fused tensor_tensor_reduce
    (max, max) over (even PSUM half, odd SBUF half) -> per-(row, doc) max
  - sum over the 32 query tokens of each query b: tiny PE matmul against a
    block-indicator matrix, accumulated in PSUM across the 16 row-tiles
  - divide by per-query token count (computed on device from q[:, :, 0])
Host gathers the 8 per-core (64, 8) score shards into the full (64, 64)
matrix and finishes the (tiny) cross-entropy.
"""

import numpy as np

B, NTOK, DIM = 64, 32, 128
C, S = 64, 1024
NCORES = 8
CSHARD = C // NCORES              # 8 docs per core
ROWS = B * NTOK                   # 2048 score rows
MTILES = ROWS // 128              # 16
DCOLS = CSHARD * S                # 8192 doc-token columns per core
TEMPERATURE = 0.02

_CACHE = {}


def _register_ttmax():
    """Custom DVE op: out = max(in0, in1); accum_out = max(s0, max_k out).
    (The native TENSOR_TENSOR_REDUCE ISA op only has mul+sum firmware; a
    max/max TTR must go through the per-NEFF custom-DVE table.)"""
    from concourse import dve_ops as DO
    from concourse.dve_spec import Spec, Src0, Src1, C0, maxx, lower
    from concourse.dve_spec import _has_src1 as has_src1
    from concourse.dve_uop import DveOpSpec

    for o in DO.OPS:
        if o.name == "TT_MAXMAX_ANT":
            return o

    def _ref(in0, in1, c0, c1, c2):
        b = np.maximum(in0.astype(np.float32), in1)
        acc = np.maximum(
            np.asarray(c0, np.float32),
            b.reshape(b.shape[0], -1).max(axis=-1, keepdims=True),
        )
        return b, acc

    spec = Spec(body=maxx(Src0, Src1), accum=maxx, accum_init=C0, reference=_ref)
    op = DO.DveOp("TT_MAXMAX_ANT", spec, subdim=False, uops_sha={})
    DO.OPS.append(op)
    DO.CUSTOM_DVE_SPECS[op.name] = op.spec
    DO._SUB_OPCODE_FOR_NAME[op.name] = DO._CUSTOM_DVE_ROW_BASE + len(DO.OPS) - 1
    ds = DveOpSpec(
        name=op.name,
        opcode=DO.get_dve_sub_opcode(op.name),
        uops=lower(spec, ver="v3"),
        rd1_en=has_src1(spec),
    )
    op.uops_sha["v3"] = ds.sha("v3")
    return op


def _build_nc(nsum="fp32mm", red="ttr"):
    import concourse.bacc as bacc
    import concourse.tile as tile
    from concourse import mybir

    f32 = mybir.dt.float32
    bf16 = mybir.dt.bfloat16
    X = mybir.AxisListType.X
    MAX = mybir.AluOpType.max
    ttmax = _register_ttmax()

    nc = bacc.Bacc("TRN2", target_bir_lowering=False, debug=False)
    qT_d = nc.dram_tensor("qT", [DIM, ROWS], bf16, kind="ExternalInput").ap()
    dT_d = nc.dram_tensor("dT", [DIM, DCOLS], bf16, kind="ExternalInput").ap()
    q0_d = nc.dram_tensor("q0t", [B, NTOK], f32, kind="ExternalInput").ap()
    bsel_d = nc.dram_tensor("bsel", [128, 124], bf16, kind="ExternalInput").ap()
    out_d = nc.dram_tensor("part", [B, CSHARD], f32, kind="ExternalOutput").ap()

    with tile.TileContext(nc) as tc:
        with (
            tc.tile_pool(name="const", bufs=1) as cpool,
            tc.tile_pool(name="odd", bufs=4) as odd_pool,
            tc.tile_pool(name="trash", bufs=3) as trash_pool,
            tc.tile_pool(name="dmax", bufs=MTILES + 1) as dmax_pool,
            tc.tile_pool(name="small", bufs=1) as small_pool,
        ):
            qT_sb = cpool.tile([DIM, ROWS], bf16)
            dT_sb = cpool.tile([DIM, DCOLS], bf16)
            bsel_sb = cpool.tile([128, 124], bf16)
            q0_sb = cpool.tile([B, NTOK], f32)

            # smallest possible first chunks so the PE can start early; big
            # DMAs later to keep the issue queue short (each issue ~0.6us)
            wsb = cpool.tile([128, 512], bf16)
            nc.gpsimd.memset(wsb[:], 0.0)
            nc.sync.dma_start(qT_sb[:, 0:128], qT_d[:, 0:128])
            nc.sync.dma_start(dT_sb[:, 0:512], dT_d[:, 0:512])
            nc.sync.dma_start(dT_sb[:, 512:1024], dT_d[:, 512:1024])
            nc.sync.dma_start(dT_sb[:, 1024:2048], dT_d[:, 1024:2048])
            nc.gpsimd.dma_start(q0_sb[:], q0_d[:])
            nc.gpsimd.dma_start(bsel_sb[:], bsel_d[:])
            nc.gpsimd.dma_start(qT_sb[:, 128:2048], qT_d[:, 128:2048])
            for j in range(1, 4):
                nc.sync.dma_start(
                    dT_sb[:, j * 2048:(j + 1) * 2048], dT_d[:, j * 2048:(j + 1) * 2048]
                )

            # lengths: count of query tokens with q[b, n, 0] != 0 (early —
            # DVE is idle during the DMA ramp)
            nz = small_pool.tile([B, NTOK], f32)
            nc.vector.tensor_scalar(
                nz[:], q0_sb[:], 0.0, None, op0=mybir.AluOpType.not_equal
            )
            lens = small_pool.tile([B, 1], f32)
            nc.vector.tensor_reduce(lens[:], nz[:], axis=X, op=mybir.AluOpType.add)
            rlen = small_pool.tile([B, 1], f32)
            nc.vector.reciprocal(rlen[:], lens[:])

            dmaxes = []
            with (
                tc.tile_pool(name="psd", bufs=3, space="PSUM") as psd_pool,
                tc.tile_pool(name="pss", bufs=1, space="PSUM") as pss_pool,
                tc.tile_pool(name="wps", bufs=1, space="PSUM") as wps_pool,
            ):
                # HAM warm-up: ~5us of dummy matmuls on a zeroed tile while
                # the input DMAs run, so the PE clock-gate (K=4/8 cold state)
                # releases before the first real matmul issues.
                warm_ps = wps_pool.tile([128, 512], f32)
                for _ in range(12):
                    nc.tensor.matmul(
                        warm_ps[:], wsb[:, 0:128], wsb[:], start=True, stop=True
                    )

                scores_ps = pss_pool.tile([B, CSHARD], f32)
                pending = []  # n-sum lags 2 M-tiles so the PE never
                # head-of-line blocks on the DVE finishing a tile's maxes
                for m in range(MTILES):
                    lhsT = qT_sb[:, m * 128:(m + 1) * 128]
                    dmax = dmax_pool.tile([128, CSHARD], bf16, tag=f"dmax{m}")
                    dmaxes.append(dmax)
                    for c in range(CSHARD):
                        ps = psd_pool.tile([128, 2, 512], f32, tag="psd")
                        for h in range(2):
                            col = c * 1024 + h * 512
                            nc.tensor.matmul(
                                ps[:, h, :],
                                lhsT,
                                dT_sb[:, col:col + 512],
                                start=True,
                                stop=True,
                            )
                        if red == "ttr":
                            osb = odd_pool.tile([128, 512], f32, tag="odd")
                            nc.scalar.copy(osb[:], ps[:, 1, :])
                            tr = trash_pool.tile([128, 512], f32, tag="trash")
                            nc.vector._custom_dve(
                                ttmax,
                                out=tr[:],
                                accum_out=dmax[:, c:c + 1],
                                in0=ps[:, 0, :],
                                in1=osb[:],
                                s0=-3.0e38,
                            )
                        else:
                            nc.vector.tensor_reduce(
                                dmax[:, c:c + 1],
                                ps[:, :, :],
                                axis=mybir.AxisListType.XY,
                                op=MAX,
                            )
                    if nsum != "none":
                        pending.append((dmax, m))
                        if len(pending) > 2:
                            pdm, pm = pending.pop(0)
                            nc.tensor.matmul(
                                scores_ps[:],
                                bsel_sb[:, 60 - 4 * pm:124 - 4 * pm],
                                pdm[:],
                                start=(pm == 0),
                                stop=False,
                            )
                if nsum == "none":
                    # debug probe: ship the last M-tile's raw doc-maxes
                    nc.sync.dma_start(out_d[:], dmaxes[-1][0:B, :])
                else:
                    for pdm, pm in pending:
                        nc.tensor.matmul(
                            scores_ps[:],
                            bsel_sb[:, 60 - 4 * pm:124 - 4 * pm],
                            pdm[:],
                            start=(pm == 0),
                            stop=(pm == MTILES - 1),
                        )
                    sc2 = small_pool.tile([B, CSHARD], f32)
                    nc.vector.tensor_scalar_mul(sc2[:], scores_ps[:], rlen[:])
                    nc.sync.dma_start(out_d[:], sc2[:])

    nc.compile()
    return nc


def _host_inputs(q, d):
    import ml_dtypes

    bf = ml_dtypes.bfloat16
    qT = np.ascontiguousarray(q.transpose(2, 0, 1).reshape(DIM, ROWS)).astype(bf)
    q0t = np.ascontiguousarray(q[:, :, 0])
    p = np.arange(128)
    bsel = np.zeros((128, 124), np.float32)
    bsel[p, 60 + p // 32] = 1.0
    bsel = bsel.astype(bf)
    in_maps = []
    for k in range(NCORES):
        dTk = np.ascontiguousarray(
            d[k * CSHARD:(k + 1) * CSHARD].transpose(2, 0, 1).reshape(DIM, DCOLS)
        ).astype(bf)
        in_maps.append({"qT": qT, "dT": dTk, "q0t": q0t, "bsel": bsel})
    return in_maps


def _loss_from_scores(S_mat, offset):
    # S_mat: (64, 64) length-normalized MaxSim scores; CE along docs axis
    logits = (S_mat.astype(np.float64)) / TEMPERATURE
    m = logits.max(axis=1, keepdims=True)
    logp = logits - m - np.log(np.exp(logits - m).sum(axis=1, keepdims=True))
    labels = np.arange(B) + offset
    return np.float32(-np.mean(logp[np.arange(B), labels]))


def kernel(**inputs):
    from concourse import bass_utils

    q = np.ascontiguousarray(np.asarray(inputs["query_embeddings"], dtype=np.float32))
    d = np.ascontiguousarray(np.asarray(inputs["doc_embeddings"], dtype=np.float32))
    offset = int(np.asarray(inputs["offset"]))
    assert q.shape == (B, NTOK, DIM) and d.shape == (C, S, DIM)

    if "nc" not in _CACHE:
        _CACHE["nc"] = _build_nc()
    nc = _CACHE["nc"]

    in_maps = _host_inputs(q, d)
    res = bass_utils.run_bass_kernel_spmd(nc, in_maps, core_ids=list(range(NCORES)))
    S_mat = np.concatenate(
        [res.results[k]["part"] for k in range(NCORES)], axis=1
    )
    return _loss_from_scores(S_mat, offset)

